# revision 5
# baseline (speedup 1.0000x reference)
"""AffinityBasedAveraging Trainium2 kernel — fp16, wide-tile, DMA-fold version.

Same math as the baseline (softmax over 9 offsets, weighted average of
shifted embeddings) but all device tensors are float16:
  - DVE tensor_tensor ops hit the 2x_1p perf mode (2 results/cycle) for
    operands that are 16-bit, stride-1, 4B-aligned. The ox=0 taps read E
    at an odd element offset (2B) and stay at 1x — unavoidable parity.
  - All DMA traffic (aff in, emb in, out, folds) halves.
Host pre-casts inputs to f16 and upcasts the f16 output back to f32;
expected rel err ~1e-3 vs the 2e-2 gate.

Sharding: 8 cores = 4 batches x 2 H-halves (as baseline).
"""

import numpy as np

import bass_rust
import concourse.bass as bass
import concourse.mybir as mybir
import concourse.tile as tile
from concourse.bass_utils import run_bass_kernel_spmd

F16 = mybir.dt.float16
F32 = mybir.dt.float32
AF = mybir.ActivationFunctionType
OP = mybir.AluOpType
AX = mybir.AxisListType

B, C, H, W = 4, 16, 512, 512
K = 9
OFFSETS = [(-1, -1), (-1, 0), (-1, 1), (0, -1), (0, 0), (0, 1), (1, -1), (1, 0), (1, 1)]
N_CORES = 8
HH = H // 2
YT = 128

_wsplit_ctr = [0]


def _split_multi_waits(nc):
    """Walrus here rejects >1 semaphore wait per instruction; split extras
    into same-engine NoOp prefixes."""
    n = 0
    for f in nc.m.functions:
        for bb in f.blocks:
            insts = bb.instructions
            if not any(
                i.sync_info is not None and len(i.sync_info.on_wait or []) > 1
                for i in insts
            ):
                continue
            new = []
            for inst in insts:
                si = inst.sync_info
                waits = list(si.on_wait) if si is not None and si.on_wait else []
                if len(waits) > 1:
                    for w in waits[:-1]:
                        _wsplit_ctr[0] += 1
                        nop = mybir.InstNoOp(name=f"I-wsplit-{_wsplit_ctr[0]}")
                        nop.engine = inst.engine
                        nop.sync_info = bass_rust.SyncInfo(on_wait=[w], on_update=[])
                        new.append(nop)
                        n += 1
                    inst.sync_info = bass_rust.SyncInfo(
                        on_wait=[waits[-1]], on_update=list(si.on_update or [])
                    )
                new.append(inst)
            insts[:] = new
    return n


def build_nc(
    split_waits=True,
    reps=1,
    dma_folds=3,
    xc=256,
    xe=512,
    e_shift_mode="hbm3x",
    tree_sum=True,
    ebufs=1,
    tmpbufs=2,
    accbufs=2,
    xbufs=2,
    sbufs=1,
    chains=1,
    hw_loop=False,
    sink_out=False,
):
    import contextlib

    nc = bass.Bass("TRN2", target_bir_lowering=False, debug=False, num_devices=N_CORES)
    aff = nc.declare_dram_parameter("aff", [K, HH, W], F16, isOutput=False)
    emb = nc.declare_dram_parameter("emb", [C, HH + 2, W + 2], F16, isOutput=False)
    if sink_out:
        out = nc.declare_dram_parameter("out", [C, HH, W], F16, isOutput=False)
        tok = nc.declare_dram_parameter("tok", [128, 16], F32, isOutput=True)
    else:
        out = nc.declare_dram_parameter("out", [C, HH, W], F16, isOutput=True)

    with tile.TileContext(nc) as tc:
        with (
            nc.allow_low_precision(reason="f16 kernel; tolerance gate is 2e-2"),
            tc.tile_pool(name="p_a", bufs=2) as p_a,
            tc.tile_pool(name="p_x", bufs=xbufs) as p_x,
            tc.tile_pool(name="p_e", bufs=ebufs) as p_e,
            tc.tile_pool(name="p_acc", bufs=accbufs) as p_acc,
            tc.tile_pool(name="p_tmp", bufs=tmpbufs) as p_tmp,
            tc.tile_pool(name="p_s", bufs=sbufs) as p_s,
        ):
            if sink_out:
                tokt = p_s.tile([128, 16], F32, tag="tok")
                nc.vector.memset(tokt[:], 1.0)
                nc.sync.dma_start(out=tok[:, :], in_=tokt[:])
            rep_iter = (
                tc.For_i(0, reps) if hw_loop else contextlib.nullcontext(range(reps))
            )
            with rep_iter as _it:
             for _rep in ([0] if hw_loop else range(reps)):
              for ty in range(HH // YT):
                ys = ty * YT
                A = p_a.tile([YT, K, W], F16, tag="A")
                nc.sync.dma_start(
                    out=A[:],
                    in_=aff[:, ys : ys + YT, :].rearrange("k y x -> y k x"),
                )
                X = p_x.tile([YT, K, W], F16, tag="X")
                nc.scalar.activation(X[:], A[:], AF.Exp)
                t4 = p_s.tile([YT, 4, W], F16, tag="t4")
                nc.vector.tensor_tensor(t4[:], X[:, 0:4, :], X[:, 4:8, :], OP.add)
                t2 = p_s.tile([YT, 2, W], F16, tag="t2")
                nc.vector.tensor_tensor(t2[:], t4[:, 0:2, :], t4[:, 2:4, :], OP.add)
                t1 = p_s.tile([YT, 1, W], F16, tag="t1")
                nc.vector.tensor_tensor(t1[:], t2[:, 0:1, :], t2[:, 1:2, :], OP.add)
                S = p_s.tile([YT, 1, W], F16, tag="S")
                nc.vector.tensor_tensor(S[:], t1[:], X[:, 8:9, :], OP.add)
                R = p_s.tile([YT, 1, W], F16, tag="R")
                nc.vector.reciprocal(R[:], S[:])
                Wfull = p_x.tile([YT, K, W], F16, tag="W")
                nc.vector.tensor_tensor(
                    Wfull[:], X[:], R[:, 0:1, :].to_broadcast((YT, K, W)), OP.mult
                )
                for xh in range(W // xc):
                    xs = xh * xc
                    Wt = Wfull[:, :, xs : xs + xc]

                    if e_shift_mode == "hbm3x":
                        xew = xc if xe is None else xe
                        if xh % (xew // xc) == 0:
                            e_tiles = {}
                            xes = (xs // xew) * xew
                            for oy in (-1, 0, 1):
                                t = p_e.tile([YT, C, xew + 2], F16, tag=f"E{oy}")
                                rs = ys + oy + 1
                                nc.sync.dma_start(
                                    out=t[:],
                                    in_=emb[
                                        :, rs : rs + YT, xes : xes + xew + 2
                                    ].rearrange("c y x -> y c x"),
                                )
                                e_tiles[oy] = t
                            build_nc._e_tiles = e_tiles
                        eoff = xs % xew
                        E = {
                            oy: build_nc._e_tiles[oy][:, :, eoff : eoff + xc + 2]
                            for oy in (-1, 0, 1)
                        }
                    else:
                        t0 = p_e.tile([YT, C, xc + 2], F16, tag="E0")
                        nc.sync.dma_start(
                            out=t0[:],
                            in_=emb[:, ys + 1 : ys + 1 + YT, xs : xs + xc + 2].rearrange(
                                "c y x -> y c x"
                            ),
                        )
                        tm = p_e.tile([YT, C, xc + 2], F16, tag="E-1")
                        nc.sync.dma_start(out=tm[1:YT], in_=t0[0 : YT - 1])
                        nc.sync.dma_start(
                            out=tm[0:1],
                            in_=emb[:, ys : ys + 1, xs : xs + xc + 2].rearrange(
                                "c y x -> y c x"
                            ),
                        )
                        tp = p_e.tile([YT, C, xc + 2], F16, tag="E1")
                        nc.sync.dma_start(out=tp[0 : YT - 1], in_=t0[1:YT])
                        nc.sync.dma_start(
                            out=tp[YT - 1 : YT],
                            in_=emb[
                                :, ys + YT + 1 : ys + YT + 2, xs : xs + xc + 2
                            ].rearrange("c y x -> y c x"),
                        )
                        E = {-1: tm, 0: t0, 1: tp}

                    acc = p_acc.tile([YT, C, xc], F16, tag="acc")
                    tmp = p_tmp.tile([YT, C, xc], F16, tag="tmp")

                    def tap(k):
                        oy, ox = OFFSETS[k]
                        wk = Wt[:, k, :][:, None, :].to_broadcast((YT, C, xc))
                        return wk, E[oy][:, :, 1 + ox : 1 + ox + xc]

                    n_dve_taps = K - (dma_folds + 1 if dma_folds else 0)
                    if dma_folds:
                        folds = []
                        for j in range(dma_folds + 1):
                            ft = p_acc.tile([YT, C, xc], F16, tag=f"fold{j}")
                            wk, ek = tap(n_dve_taps + j)
                            nc.vector.tensor_tensor(ft[:], wk, ek, OP.mult)
                            folds.append(ft)
                        for j in range(1, dma_folds + 1):
                            for xq in range(0, xc, 128):
                                nc.gpsimd.dma_start(
                                    out=folds[0][:, :, xq : xq + 128],
                                    in_=folds[j][:, :, xq : xq + 128],
                                    accum_op=OP.add,
                                )
                    if chains == 2 and n_dve_taps >= 4:
                        acc1 = p_acc.tile([YT, C, xc], F16, tag="acc1")
                        tmp1 = p_tmp.tile([YT, C, xc], F16, tag="tmp1")
                        ka = [k for k in range(n_dve_taps) if k % 2 == 0]
                        kb = [k for k in range(n_dve_taps) if k % 2 == 1]
                        # interleave two independent mult/add chains
                        for i in range(max(len(ka), len(kb))):
                            for ks, a_t, t_t in ((ka, acc, tmp), (kb, acc1, tmp1)):
                                if i >= len(ks):
                                    continue
                                wk, ek = tap(ks[i])
                                if i == 0:
                                    nc.vector.tensor_tensor(a_t[:], wk, ek, OP.mult)
                                else:
                                    nc.vector.tensor_tensor(t_t[:], wk, ek, OP.mult)
                                    nc.vector.tensor_tensor(
                                        a_t[:], a_t[:], t_t[:], OP.add
                                    )
                        nc.vector.tensor_tensor(acc[:], acc[:], acc1[:], OP.add)
                    else:
                        for k in range(n_dve_taps):
                            wk, ek = tap(k)
                            if k == 0:
                                nc.vector.tensor_tensor(acc[:], wk, ek, OP.mult)
                            else:
                                nc.vector.tensor_tensor(tmp[:], wk, ek, OP.mult)
                                nc.vector.tensor_tensor(acc[:], acc[:], tmp[:], OP.add)
                    if dma_folds:
                        for xq in range(0, xc, 128):
                            nc.gpsimd.dma_start(
                                out=acc[:, :, xq : xq + 128],
                                in_=folds[0][:, :, xq : xq + 128],
                                accum_op=OP.add,
                            )

                    nc.sync.dma_start(
                        out=out[:, ys : ys + YT, xs : xs + xc].rearrange(
                            "c y x -> y c x"
                        ),
                        in_=acc[:],
                    )

    if split_waits:
        _split_multi_waits(nc)
    return nc


_nc_cache = None


def _get_nc():
    global _nc_cache
    if _nc_cache is None:
        _nc_cache = build_nc()
    return _nc_cache


def shard_inputs(affinities, embedding):
    """Full f32 inputs -> 8 per-core f16 input maps (batch x H-half)."""
    affinities = np.asarray(affinities)
    embedding = np.asarray(embedding)
    ycl = lambda idx: np.clip(idx, 0, H - 1)
    xcl = np.clip(np.arange(-1, W + 1), 0, W - 1)
    in_maps = []
    for i in range(N_CORES):
        b, half = i // 2, i % 2
        y0 = half * HH
        aff_s = np.ascontiguousarray(
            affinities[b, :, y0 : y0 + HH, :].astype(np.float16)
        )
        rows = ycl(np.arange(y0 - 1, y0 + HH + 1))
        emb_s = np.ascontiguousarray(embedding[b][:, rows][:, :, xcl].astype(np.float16))
        in_maps.append({"aff": aff_s, "emb": emb_s})
    return in_maps


def unshard_outputs(results):
    out = np.empty((B, C, H, W), np.float32)
    for i in range(N_CORES):
        b, half = i // 2, i % 2
        y0 = half * HH
        out[b, :, y0 : y0 + HH, :] = results[i]["out"].astype(np.float32)
    return out


def kernel(affinities, embedding):
    nc = _get_nc()
    in_maps = shard_inputs(affinities, embedding)
    try:
        res = run_bass_kernel_spmd(nc, in_maps, list(range(N_CORES)))
    except Exception:
        import time as _t

        _t.sleep(2.0)
        res = run_bass_kernel_spmd(nc, in_maps, list(range(N_CORES)))
    out = unshard_outputs(res.results)
    kernel.last_result = res
    return out
